# revision 1
# baseline (speedup 1.0000x reference)
"""BiLSTM-CRF loss kernel for 8 Trainium2 NeuronCores (data-parallel over batch).

Self-contained: hardcodes all shapes from the problem spec.
Returns scalar f32 loss (mean over batch of CRF NLL).

Math reformulation (validated vs reference at ~1e-7 rel):
 - LSTM gates via one tanh (sigmoid(x) = 0.5 tanh(x/2) + 0.5); i,f,o weight
   rows pre-halved on host. States kept as c' = 2c, h' = 2h (weights absorb).
 - Reverse-direction masking: add -30000 to i,f,o pre-activations at padded
   steps (forces sigmoids to exactly 0 => state resets, matching the
   reference's mask multiply). Forward direction needs no masking: padded
   outputs are garbage but every downstream read is masked/selected.
 - LayerNorm folded into the feature matmul: feats = rstd*(Wg h - wsum*muN)
   + c0 with Wg = W_lin*gamma, mu/var from PE ones-matmul reductions
   (hidden dim lives on partitions).
 - CRF in exp space: w_t = exp(alpha_t) * kappa^t with kappa folded into the
   transition matrix; no per-step normalization (log drift fits f32 range).
   alpha at sen_len recovered from the END-transition readout row of the
   per-step matmul, stored per step, gathered at sen_len.
"""

import numpy as np
import ml_dtypes

VOCAB, EMBD, HID, K = 100000, 50, 200, 32
H = 100
START, END = 30, 31
B, T = 512, 256
NCORES = 8
BC = B // NCORES            # 64 sequences per core
LN_EPS = 1e-5
KLOG = 4.9                  # -log(kappa)

bf16 = ml_dtypes.bfloat16

_PROGRAM_CACHE = {}
DEBUG_DUMP = False


def _dims(Tn):
    NT = Tn * BC
    return dict(
        NT=NT,
        CH=NT // 512,
        NG=max(1, NT // 4096),
        GSZ=NT // max(1, NT // 4096),
        NQ=4 if Tn >= 32 else 1,
        QT=Tn // (4 if Tn >= 32 else 1),
        R4=max(1, NT // 4096),
        L4=NT // max(1, NT // 4096),
        L4U=-(-((NT + BC) // max(1, NT // 4096)) // BC) * BC,
    )


def _build_program(Tn):
    import concourse.bass as bass
    import concourse.bacc as bacc
    import concourse.mybir as mybir
    import concourse.tile as tile
    from concourse.alu_op_type import AluOpType as op
    from concourse.masks import make_identity
    from contextlib import ExitStack

    dt = mybir.dt
    AF = mybir.ActivationFunctionType
    D = _dims(Tn)
    NT, CH, NG, GSZ = D["NT"], D["CH"], D["NG"], D["GSZ"]
    NQ, QT = D["NQ"], D["QT"]
    R4, L4, L4U = D["R4"], D["L4"], D["L4U"]
    RP = (R4 - 1) * 32 + 1       # partition extent of 4-row buffers
    PKP = 128 // R4              # partitions per row after packing
    PKC = NT // 128              # packed free size

    nc = bacc.Bacc()

    d_emb = nc.dram_tensor("emb_tab", [VOCAB, EMBD], dt.bfloat16, kind="ExternalInput")
    d_gidx = nc.dram_tensor("gidx", [128, NT // 128], dt.int32, kind="ExternalInput")
    d_wx = {dn: nc.dram_tensor(f"wx_{dn}", [EMBD + 2, 4 * H], dt.bfloat16, kind="ExternalInput")
            for dn in "fb"}
    d_wh = {dn: nc.dram_tensor(f"wh_{dn}", [H, 4 * H], dt.bfloat16, kind="ExternalInput")
            for dn in "fb"}
    d_invm = nc.dram_tensor("invm", [1, NT], dt.bfloat16, kind="ExternalInput")
    d_ones = nc.dram_tensor("ones_row", [1, NT], dt.bfloat16, kind="ExternalInput")
    d_w0 = nc.dram_tensor("w0", [K, BC], dt.float32, kind="ExternalInput")
    d_wgf = nc.dram_tensor("wgt_f", [H, K], dt.bfloat16, kind="ExternalInput")
    d_wgb = nc.dram_tensor("wgt_b", [H, K], dt.bfloat16, kind="ExternalInput")
    d_nws = nc.dram_tensor("negwsum", [1, K], dt.bfloat16, kind="ExternalInput")
    d_c0 = nc.dram_tensor("c0col", [K, 1], dt.float32, kind="ExternalInput")
    d_mmat = nc.dram_tensor("mmat", [K, K + 1], dt.float32, kind="ExternalInput")
    d_trT = nc.dram_tensor("transT", [K, K], dt.bfloat16, kind="ExternalInput")
    d_tend = nc.dram_tensor("trans_end", [K, 1], dt.bfloat16, kind="ExternalInput")
    d_c0b = nc.dram_tensor("c0bf", [K, 1], dt.bfloat16, kind="ExternalInput")
    d_ohe = nc.dram_tensor("oh_end", [K, BC], dt.bfloat16, kind="ExternalInput")
    d_ohem = nc.dram_tensor("oh_em", [K, NT], dt.bfloat16, kind="ExternalInput")
    d_ohpr = nc.dram_tensor("oh_prev", [K, NT], dt.bfloat16, kind="ExternalInput")
    d_ui = nc.dram_tensor("u_idx", [BC, 1], dt.int32, kind="ExternalInput")
    d_lenk = nc.dram_tensor("len_klog", [BC, 1], dt.float32, kind="ExternalInput")
    d_loss = nc.dram_tensor("loss", [BC, 1], dt.float32, kind="ExternalOutput")
    if DEBUG_DUMP:
        d_dbg_hf = nc.dram_tensor("dbg_hf", [H, NT], dt.bfloat16, kind="ExternalOutput")
        d_dbg_hb = nc.dram_tensor("dbg_hb", [H, NT], dt.bfloat16, kind="ExternalOutput")
        d_dbg_e = nc.dram_tensor("dbg_e", [K * NG, GSZ], dt.float32, kind="ExternalOutput")
        d_dbg_u = nc.dram_tensor("dbg_u", [R4, L4U], dt.float32, kind="ExternalOutput")
        d_dbg_rs = nc.dram_tensor("dbg_rs", [1, BC], dt.float32, kind="ExternalOutput")
        d_dbg_rr = nc.dram_tensor("dbg_rr", [1, BC], dt.float32, kind="ExternalOutput")
        d_dbg_rc = nc.dram_tensor("dbg_rc", [BC, 1], dt.float32, kind="ExternalOutput")
        d_dbg_tot = nc.dram_tensor("dbg_tot", [BC, 1], dt.float32, kind="ExternalOutput")

    with tile.TileContext(nc) as tc, ExitStack() as ctx:
        const = ctx.enter_context(tc.tile_pool(name="const", bufs=1))
        big = ctx.enter_context(tc.tile_pool(name="big", bufs=1))
        dramp = ctx.enter_context(tc.tile_pool(name="dramp", bufs=1, space="DRAM"))
        goldps = ctx.enter_context(tc.tile_pool(name="goldps", bufs=1, space="PSUM"))

        u_d = dramp.tile([R4 * L4U, 1], dt.float32, tag="u_d")
        r_d = dramp.tile([BC, 1], dt.float32, tag="r_d")

        ident = const.tile([128, 128], dt.bfloat16)
        make_identity(nc, ident[:])
        wx = {dn: const.tile([EMBD + 2, 4 * H], dt.bfloat16, tag=f"wx{dn}", name=f"wx{dn}") for dn in "fb"}
        wh = {dn: const.tile([H, 4 * H], dt.bfloat16, tag=f"wh{dn}", name=f"wh{dn}") for dn in "fb"}
        for dn in "fb":
            nc.sync.dma_start(wx[dn][:], d_wx[dn][:])
            nc.sync.dma_start(wh[dn][:], d_wh[dn][:])
        wgf = const.tile([H, K], dt.bfloat16)
        wgb = const.tile([H, K], dt.bfloat16)
        nc.sync.dma_start(wgf[:], d_wgf[:])
        nc.sync.dma_start(wgb[:], d_wgb[:])
        nws = const.tile([1, K], dt.bfloat16)
        nc.sync.dma_start(nws[:], d_nws[:])
        c0col = const.tile([K, 1], dt.float32)
        nc.sync.dma_start(c0col[:], d_c0[:])
        mmat = const.tile([K, K + 1], dt.float32)
        nc.sync.dma_start(mmat[:], d_mmat[:])
        ones100 = const.tile([H, 1], dt.bfloat16)
        nc.vector.memset(ones100[:], 1.0)
        ones1k = const.tile([1, K], dt.bfloat16)
        nc.vector.memset(ones1k[:], 1.0)
        ones1kf = const.tile([K, 1], dt.bfloat16)
        nc.vector.memset(ones1kf[:], 1.0)


        hq = {dn: [big.tile([H, QT * BC], dt.bfloat16, tag=f"h{dn}{q}", name=f"h{dn}{q}") for q in range(NQ)]
              for dn in "fb"}
        epk = [big.tile([K, GSZ], dt.float32, tag=f"epk{q}", name=f"epk{q}")
               for q in range(NG)]
        fpk = [big.tile([K, GSZ], dt.bfloat16, tag=f"fpk{q}", name=f"fpk{q}")
               for q in range(NG)]

        # ================ P0: embedding gather + transpose ================
        ctx01 = ExitStack()
        p01 = ctx01.enter_context(tc.tile_pool(name="p01", bufs=1))
        xT = p01.tile([EMBD + 2, NT], dt.bfloat16, tag="xT")
        nc.sync.dma_start(xT[EMBD:EMBD + 1, :], d_ones[:])
        nc.sync.dma_start(xT[EMBD + 1:EMBD + 2, :], d_invm[:])
        with tc.tile_pool(name="p0", bufs=3) as p0, \
             tc.tile_pool(name="p0ps", bufs=2, space="PSUM") as p0ps:
            gidx = const.tile([128, NT // 128], dt.int32)
            nc.sync.dma_start(gidx[:], d_gidx[:])
            NJ = NT // 128
            # batches of 4 calls (512 tokens); alternate front (fwd) and back
            # (bwd) so both LSTM directions can start while gathering.
            batches = []
            nb = NJ // 4
            for k in range((nb + 1) // 2):
                batches.append(k)
                if nb - 1 - k > k:
                    batches.append(nb - 1 - k)
            for bi in batches:
                j0 = bi * 4
                xg = p0.tile([128, 4 * EMBD], dt.bfloat16, tag="xg")
                tp = p0ps.tile([EMBD, 4 * 128], dt.bfloat16, tag="tp")
                for qq in range(4):
                    nc.gpsimd.indirect_dma_start(
                        out=xg[:, qq * EMBD:(qq + 1) * EMBD],
                        out_offset=None,
                        in_=d_emb[:],
                        in_offset=bass.IndirectOffsetOnAxis(
                            ap=gidx[:, j0 + qq:j0 + qq + 1], axis=0),
                    )
                    nc.tensor.matmul(
                        out=tp[:, qq * 128:(qq + 1) * 128],
                        lhsT=xg[:, qq * EMBD:(qq + 1) * EMBD],
                        rhs=ident[:], is_transpose=True,
                        start=(qq == 0), stop=(qq == 3),
                    )
                dst = j0 * 128
                if bi % 2 == 0:
                    nc.vector.tensor_copy(out=xT[0:EMBD, dst:dst + 512], in_=tp[:])
                else:
                    nc.scalar.copy(out=xT[0:EMBD, dst:dst + 512], in_=tp[:])

        # ================ P1: the two LSTMs ================
        with tc.tile_pool(name="p1", bufs=2) as p1, \
             tc.tile_pool(name="p1s", bufs=1) as p1s, \
             tc.tile_pool(name="p1ps", bufs=2, space="PSUM") as p1ps:
            cst = {dn: p1s.tile([H, BC], dt.float32, tag=f"c{dn}", name=f"c{dn}") for dn in "fb"}

            def lstm_step(dn, t, prev_t, first):
                ps = p1ps.tile([H, 4 * BC], dt.float32, tag=f"g{dn}")
                rx = xT[:, t * BC:(t + 1) * BC]
                n_mm = 4 + (0 if first else 4)
                k_mm = 0
                def mm(o_, l_, r_):
                    nonlocal k_mm
                    nc.tensor.matmul(out=o_, lhsT=l_, rhs=r_, start=(k_mm == 0),
                                     stop=(k_mm == n_mm - 1))
                    k_mm += 1
                for g in range(4):
                    mm(ps[:, g * BC:(g + 1) * BC], wx[dn][:, g * H:(g + 1) * H], rx)
                if not first:
                    pq, pc = prev_t // QT, (prev_t % QT) * BC
                    rh = hq[dn][pq][:, pc:pc + BC]
                    for g in range(4):
                        mm(ps[:, g * BC:(g + 1) * BC], wh[dn][:, g * H:(g + 1) * H], rh)
                G = p1.tile([H, 4 * BC], dt.float32, tag=f"G{dn}")
                nc.scalar.activation(out=G[:], in_=ps[:], func=AF.Tanh)
                th_i, th_f = G[:, 0:BC], G[:, BC:2 * BC]
                th_o, th_g = G[:, 2 * BC:3 * BC], G[:, 3 * BC:4 * BC]
                c = cst[dn]
                u = p1.tile([H, BC], dt.float32, tag=f"u{dn}")
                nc.vector.scalar_tensor_tensor(out=u[:], in0=th_i, scalar=1.0,
                                               in1=th_g, op0=op.add, op1=op.mult)
                if first:
                    nc.vector.tensor_copy(out=c[:], in_=u[:])
                else:
                    sf = p1.tile([H, BC], dt.float32, tag=f"sf{dn}")
                    nc.vector.tensor_scalar(out=sf[:], in0=th_f, scalar1=0.5,
                                            scalar2=0.5, op0=op.mult, op1=op.add)
                    v = p1.tile([H, BC], dt.float32, tag=f"v{dn}")
                    nc.vector.tensor_tensor(out=v[:], in0=sf[:], in1=c[:], op=op.mult)
                    nc.vector.tensor_tensor(out=c[:], in0=v[:], in1=u[:], op=op.add)
                thc = p1.tile([H, BC], dt.float32, tag=f"thc{dn}")
                nc.scalar.activation(out=thc[:], in_=c[:], func=AF.Tanh, scale=0.5)
                qh, ch_ = t // QT, (t % QT) * BC
                nc.vector.scalar_tensor_tensor(
                    out=hq[dn][qh][:, ch_:ch_ + BC], in0=th_o, scalar=1.0,
                    in1=thc[:], op0=op.add, op1=op.mult)

            for s in range(Tn):
                lstm_step("f", s, s - 1, s == 0)
                lstm_step("b", Tn - 1 - s, Tn - s, s == 0)

        ctx01.close()  # free xT
        # ================ P2: E-feats build ================
        with tc.tile_pool(name="p2", bufs=3) as p2, \
             tc.tile_pool(name="p2s", bufs=1) as p2s, \
             tc.tile_pool(name="p2ps", bufs=1, space="PSUM") as p2ps, \
             tc.tile_pool(name="p2pg", bufs=2, space="PSUM") as p2pg:

            mupk = p2s.tile([128, PKC], dt.float32, tag="mupk")
            msqpk = p2s.tile([128, PKC], dt.float32, tag="msqpk")
            for c in range(CH):
                psmu = p2ps.tile([1, 512], dt.float32, tag="psmu")
                psmsq = p2ps.tile([1, 512], dt.float32, tag="psmsq")
                q, off = (c * 512) // (QT * BC), (c * 512) % (QT * BC)
                hfc = hq["f"][q][:, off:off + 512]
                hbc = hq["b"][q][:, off:off + 512]
                hsqf = p2.tile([H, 512], dt.bfloat16, tag="hsqf")
                nc.vector.tensor_tensor(out=hsqf[:], in0=hfc, in1=hfc, op=op.mult)
                hsqb = p2.tile([H, 512], dt.bfloat16, tag="hsqb")
                nc.vector.tensor_tensor(out=hsqb[:], in0=hbc, in1=hbc, op=op.mult)
                nc.tensor.matmul(out=psmu[:], lhsT=ones100[:], rhs=hfc,
                                 start=True, stop=False)
                nc.tensor.matmul(out=psmu[:], lhsT=ones100[:], rhs=hbc,
                                 start=False, stop=True)
                nc.tensor.matmul(out=psmsq[:], lhsT=ones100[:],
                                 rhs=hsqf[:], start=True, stop=False)
                nc.tensor.matmul(out=psmsq[:], lhsT=ones100[:],
                                 rhs=hsqb[:], start=False, stop=True)
                stgmu = p2.tile([1, 512], dt.float32, tag="stgmu")
                nc.scalar.copy(out=stgmu[:], in_=psmu[:])
                stgmsq = p2.tile([1, 512], dt.float32, tag="stgmsq")
                nc.vector.tensor_copy(out=stgmsq[:], in_=psmsq[:])
                nc.sync.dma_start(out=mupk[4 * c:4 * c + 4, :], in_=stgmu[:])
                nc.sync.dma_start(out=msqpk[4 * c:4 * c + 4, :], in_=stgmsq[:])
            sq = p2s.tile([128, PKC], dt.float32, tag="sqpk")
            nc.vector.scalar_tensor_tensor(out=sq[:], in0=mupk[:], scalar=1.0 / 160000.0,
                                           in1=mupk[:], op0=op.mult, op1=op.mult)
            var = p2s.tile([128, PKC], dt.float32, tag="varpk")
            nc.vector.scalar_tensor_tensor(out=var[:], in0=msqpk[:], scalar=1.0 / 800.0,
                                           in1=sq[:], op0=op.mult, op1=op.subtract)
            epsc = p2s.tile([128, 1], dt.float32, tag="epsc")
            nc.vector.memset(epsc[:], LN_EPS)
            lnv = p2s.tile([128, PKC], dt.float32, tag="lnvpk")
            nc.scalar.activation(out=lnv[:], in_=var[:], func=AF.Ln, bias=epsc[:])
            rstdpk = p2s.tile([128, PKC], dt.bfloat16, tag="rstdpk")
            nc.scalar.activation(out=rstdpk[:], in_=lnv[:], func=AF.Exp, scale=-0.5)
            spk = p2s.tile([128, PKC], dt.bfloat16, tag="spk")
            nc.vector.tensor_copy(out=spk[:], in_=mupk[:])

            trT = p2s.tile([K, K], dt.bfloat16, tag="trT")
            nc.sync.dma_start(trT[:], d_trT[:])
            c0b = p2s.tile([K, 1], dt.bfloat16, tag="c0b")
            nc.sync.dma_start(c0b[:], d_c0b[:])
            realp = goldps.tile([1, 512], dt.float32, tag="realp")
            n_acc = 3 * CH
            k_acc = 0
            for c in range(CH):
                pg = p2pg.tile([K, 512], dt.float32, tag="pg")
                q, off = (c * 512) // (QT * BC), (c * 512) % (QT * BC)
                nc.tensor.matmul(out=pg[:], lhsT=wgf[:], rhs=hq["f"][q][:, off:off + 512],
                                 start=True, stop=False)
                nc.tensor.matmul(out=pg[:], lhsT=wgb[:], rhs=hq["b"][q][:, off:off + 512],
                                 start=False, stop=False)
                rstg = p2.tile([1, 512], dt.bfloat16, tag="rstg")
                nc.sync.dma_start(out=rstg[:], in_=rstdpk[4 * c:4 * c + 4, :])
                sstg = p2.tile([1, 512], dt.bfloat16, tag="sstg")
                nc.sync.dma_start(out=sstg[:], in_=spk[4 * c:4 * c + 4, :])
                nc.tensor.matmul(out=pg[:], lhsT=nws[:], rhs=sstg[:],
                                 start=False, stop=True)
                rb = p2pg.tile([K, 512], dt.float32, tag="rb", bufs=1)
                nc.tensor.matmul(out=rb[:], lhsT=ones1k[:], rhs=rstg[:],
                                 start=True, stop=True)
                rbs = p2.tile([K, 512], dt.bfloat16, tag="rbs")
                nc.scalar.copy(out=rbs[:], in_=rb[:])
                gq, gc = (c * 512) // GSZ, (c * 512) % GSZ
                fsl = fpk[gq][:, gc:gc + 512]
                nc.vector.tensor_tensor(out=fsl, in0=pg[:], in1=rbs[:], op=op.mult)
                nc.scalar.activation(out=epk[gq][:, gc:gc + 512], in_=fsl,
                                     func=AF.Exp, bias=c0col[:])
                # ---- gold-score accumulation for this chunk ----
                ohem = p2.tile([K, 512], dt.bfloat16, tag="ohem")
                nc.sync.dma_start(ohem[:], d_ohem[:, c * 512:(c + 1) * 512])
                ohpr = p2.tile([K, 512], dt.bfloat16, tag="ohpr")
                nc.sync.dma_start(ohpr[:], d_ohpr[:, c * 512:(c + 1) * 512])
                Rp = p2pg.tile([K, 512], dt.float32, tag="Rp", bufs=1)
                nc.tensor.matmul(out=Rp[:], lhsT=trT[:], rhs=ohem[:],
                                 start=True, stop=True)
                tsel = p2.tile([K, 512], dt.bfloat16, tag="tsel")
                nc.vector.tensor_tensor(out=tsel[:], in0=Rp[:], in1=ohpr[:], op=op.mult)
                esel = p2.tile([K, 512], dt.bfloat16, tag="esel")
                nc.vector.tensor_tensor(out=esel[:], in0=fsl, in1=ohem[:], op=op.mult)
                for rhs_ in (tsel[:], esel[:]):
                    nc.tensor.matmul(out=realp[:], lhsT=ones1kf[:], rhs=rhs_,
                                     start=(k_acc == 0), stop=(k_acc == n_acc - 1))
                    k_acc += 1
                nc.tensor.matmul(out=realp[:], lhsT=c0b[:], rhs=ohem[:],
                                 start=(k_acc == 0), stop=(k_acc == n_acc - 1))
                k_acc += 1

        # ================ P3: CRF recursion ================
        with tc.tile_pool(name="p3", bufs=1) as p3, \
             tc.tile_pool(name="p3ps", bufs=2, space="PSUM") as p3ps:
            HB = BC // 2
            wa = p3.tile([K, HB], dt.float32, tag="wa")
            wb = p3.tile([K, HB], dt.float32, tag="wb")
            nc.sync.dma_start(wa[:], d_w0[:, 0:HB])
            nc.sync.dma_start(wb[:], d_w0[:, HB:BC])
            n_oct = (Tn + 1 + 7) // 8
            for o in range(n_oct):
                t0, t1 = o * 8 + 1, min(o * 8 + 8, Tn + 1)
                ustg = p3.tile([K + 1, 512], dt.float32, tag="ustg", bufs=2)
                for t in range(t0, t1 + 1):
                    pv = p3ps.tile([K + 1, BC], dt.float32, tag="pv", bufs=4)
                    nc.tensor.matmul(out=pv[:, 0:HB], lhsT=mmat[:],
                                     rhs=wa[:], start=True, stop=False)
                    nc.tensor.matmul(out=pv[:, HB:BC], lhsT=mmat[:],
                                     rhs=wb[:], start=False, stop=True)
                    if t <= Tn:
                        tok = (t - 1) * BC
                        gq, gc = tok // GSZ, tok % GSZ
                        nc.vector.tensor_tensor(
                            out=wa[:], in0=pv[0:K, 0:HB],
                            in1=epk[gq][:, gc:gc + HB], op=op.mult)
                        nc.vector.tensor_tensor(
                            out=wb[:], in0=pv[0:K, HB:BC],
                            in1=epk[gq][:, gc + HB:gc + BC], op=op.mult)
                    so = (t - 1) % 8
                    nc.scalar.copy(out=ustg[K:K + 1, so * BC:(so + 1) * BC],
                                   in_=pv[K:K + 1, :])
                nsteps = t1 - t0 + 1
                nc.sync.dma_start(
                    out=u_d[(t0 - 1) * BC:(t0 - 1) * BC + nsteps * BC, :],
                    in_=ustg[K:K + 1, :nsteps * BC])

        if DEBUG_DUMP:
            for q in range(NQ):
                nc.sync.dma_start(d_dbg_hf[:, q * QT * BC:(q + 1) * QT * BC], hq["f"][q][:])
                nc.sync.dma_start(d_dbg_hb[:, q * QT * BC:(q + 1) * QT * BC], hq["b"][q][:])
            for q in range(NG):
                nc.sync.dma_start(d_dbg_e[:, q * GSZ:(q + 1) * GSZ], epk[q][:])
        # ================ P4: final loss ================
        with tc.tile_pool(name="p4", bufs=1) as p4, \
             tc.tile_pool(name="p4ps", bufs=1, space="PSUM") as p4ps:
            tend = p4.tile([K, 1], dt.bfloat16, tag="tend")
            nc.sync.dma_start(tend[:], d_tend[:])
            ohe = p4.tile([K, BC], dt.bfloat16, tag="ohe")
            nc.sync.dma_start(ohe[:], d_ohe[:])
            endp = p4ps.tile([1, BC], dt.float32, tag="endp")
            nc.tensor.matmul(out=endp[:], lhsT=tend[:], rhs=ohe[:],
                             start=True, stop=True)
            rsub = p4.tile([1, BC], dt.float32, tag="rsub")
            nc.vector.tensor_reduce(
                out=rsub[:], in_=realp[:].rearrange("one (t b) -> one b t", b=BC),
                axis=mybir.AxisListType.X, op=op.add)
            rrow2 = p4.tile([1, BC], dt.float32, tag="rrow2")
            nc.vector.tensor_tensor(out=rrow2[:], in0=rsub[:], in1=endp[:], op=op.add)
            nc.sync.dma_start(out=r_d[:], in_=rrow2[:])
            rcol = p4.tile([BC, 1], dt.float32, tag="rcol")
            nc.sync.dma_start(out=rcol[:], in_=r_d[:])

            ui = p4.tile([BC, 1], dt.int32, tag="ui")
            nc.sync.dma_start(ui[:], d_ui[:])
            lenk = p4.tile([BC, 1], dt.float32, tag="lenk")
            nc.sync.dma_start(lenk[:], d_lenk[:])
            ug = p4.tile([BC, 1], dt.float32, tag="ug")
            nc.gpsimd.indirect_dma_start(out=ug[:], out_offset=None, in_=u_d[:],
                                         in_offset=bass.IndirectOffsetOnAxis(ap=ui[:], axis=0))
            tot = p4.tile([BC, 1], dt.float32, tag="tot")
            nc.scalar.activation(out=tot[:], in_=ug[:], func=AF.Ln)
            nc.vector.tensor_tensor(out=tot[:], in0=tot[:], in1=lenk[:], op=op.add)
            if DEBUG_DUMP:
                nc.sync.dma_start(d_dbg_rs[:], rsub[:])
                nc.sync.dma_start(d_dbg_rr[:], rrow2[:])
                nc.sync.dma_start(d_dbg_rc[:], rcol[:])
                nc.sync.dma_start(d_dbg_tot[:], tot[:])
            lout = p4.tile([BC, 1], dt.float32, tag="lout")
            nc.vector.tensor_tensor(out=lout[:], in0=tot[:], in1=rcol[:], op=op.subtract)
            nc.sync.dma_start(out=d_loss[:], in_=lout[:])

    nc.compile()
    return nc


def _prep_core_inputs(sent, tags, slen, consts, Tn):
    """Host-side index prep for one core. sent/tags [BC,Tn] slen [BC]."""
    D = _dims(Tn)
    NT, GSZ, L4U = D["NT"], D["GSZ"], D["L4U"]

    sent_tm = np.ascontiguousarray(sent.T).reshape(-1)      # t-major tokens
    gidx = np.ascontiguousarray(sent_tm.reshape(NT // 128, 128).T).astype(np.int32)

    tgrid = np.repeat(np.arange(Tn), BC)
    bgrid = np.tile(np.arange(BC), Tn)
    invm = (tgrid >= slen[bgrid]).astype(np.float32).reshape(1, NT).astype(bf16)

    tags_ext = np.concatenate([np.full((BC, 1), START, np.int64), tags], axis=1)
    mrow = (tgrid < slen[bgrid]).astype(np.float32)          # [NT] mask, t-major
    tag_tm = tags.T.reshape(-1)                              # tag at token (t,b)
    prev_tm = tags_ext[:, :Tn].T.reshape(-1)                 # prev tag at (t,b)
    kk = np.arange(K)[:, None]
    oh_em = ((tag_tm[None, :] == kk) * mrow[None, :]).astype(bf16)
    oh_prev = ((prev_tm[None, :] == kk) * mrow[None, :]).astype(bf16)
    oh_end = (tags_ext[np.arange(BC), slen][None, :] == kk).astype(bf16)

    ui = (slen * BC + np.arange(BC)).astype(np.int32).reshape(BC, 1)

    w0 = np.zeros((K, BC), np.float32)
    w0[START, :] = 1.0

    d = dict(consts)
    d.update(dict(
        gidx=gidx,
        invm=invm,
        oh_em=np.ascontiguousarray(oh_em),
        oh_prev=np.ascontiguousarray(oh_prev),
        oh_end=np.ascontiguousarray(oh_end),
        u_idx=ui,
        w0=w0,
        len_klog=(slen * KLOG).astype(np.float32).reshape(BC, 1),
    ))
    return d


def _prep_consts(emb, Wf_ih, Wf_hh, bfv, Wb_ih, Wb_hh, bbv, gamma, beta, W_lin, trans, Tn):
    D = _dims(Tn)
    sc = np.ones((4 * H, 1), np.float32)
    sc[0:H] = 0.5
    sc[H:2 * H] = 0.5
    sc[3 * H:4 * H] = 0.5
    # reference gate order [i,f,g,o] -> device order [i,f,o,g]
    perm = np.concatenate([np.arange(0, H), np.arange(H, 2 * H),
                           np.arange(3 * H, 4 * H), np.arange(2 * H, 3 * H)])

    def mk(Wi, Wh, b, bwd):
        Wi_s, Wh_s, b_s = Wi * sc, Wh * sc * 0.5, b * sc[:, 0]
        Wi_p, Wh_p, b_p = Wi_s[perm], Wh_s[perm], b_s[perm]
        wxa = np.zeros((EMBD + 2, 4 * H), np.float32)
        wxa[:EMBD] = Wi_p.T
        wxa[EMBD] = b_p
        if bwd:
            wxa[EMBD + 1, 0:3 * H] = -30000.0   # i, f, o gate masking
        return np.ascontiguousarray(wxa).astype(bf16), \
            np.ascontiguousarray(Wh_p.T).astype(bf16)

    wx_f, wh_f = mk(Wf_ih, Wf_hh, bfv, False)
    wx_b, wh_b = mk(Wb_ih, Wb_hh, bbv, True)

    Wg = (W_lin * gamma[None, :]) * 0.5
    wsum = (W_lin * gamma[None, :]).sum(1)
    c0 = (W_lin @ beta).astype(np.float32)
    kap = np.exp(-KLOG)
    mmat = np.zeros((K, K + 1), np.float32)
    mmat[:, :K] = kap * np.exp(trans)
    mmat[:, K] = np.exp(trans[:, END])

    return dict(
        emb_tab=np.ascontiguousarray(emb).astype(bf16),
        wx_f=wx_f, wh_f=wh_f, wx_b=wx_b, wh_b=wh_b,
        ones_row=np.ones((1, D["NT"]), bf16),
        wgt_f=np.ascontiguousarray(Wg[:, :H].T).astype(bf16),
        wgt_b=np.ascontiguousarray(Wg[:, H:].T).astype(bf16),
        negwsum=np.ascontiguousarray((-(wsum / 400.0)).reshape(1, K)).astype(bf16),
        c0col=np.ascontiguousarray(c0.reshape(K, 1)),
        mmat=mmat,
        transT=np.ascontiguousarray(trans.T).astype(bf16),
        trans_end=np.ascontiguousarray(trans[:, END].reshape(K, 1)).astype(bf16),
        c0bf=np.ascontiguousarray(c0.reshape(K, 1)).astype(bf16),
    )


def kernel(sentence, tags, sen_len, emb, Wf_ih, Wf_hh, bf, Wb_ih, Wb_hh, bb,
           gamma, beta, W_lin, trans):
    from concourse import bass_utils

    sentence = np.asarray(sentence).astype(np.int64)
    tags_a = np.asarray(tags).astype(np.int64)
    slen = np.asarray(sen_len).astype(np.int64)
    fp = lambda a: np.ascontiguousarray(np.asarray(a), dtype=np.float32)

    consts = _prep_consts(fp(emb), fp(Wf_ih), fp(Wf_hh), fp(bf), fp(Wb_ih), fp(Wb_hh),
                          fp(bb), fp(gamma), fp(beta), fp(W_lin), fp(trans), T)

    if T not in _PROGRAM_CACHE:
        _PROGRAM_CACHE[T] = _build_program(T)
    nc = _PROGRAM_CACHE[T]

    in_maps = []
    for core in range(NCORES):
        b0 = core * BC
        in_maps.append(_prep_core_inputs(
            sentence[b0:b0 + BC], tags_a[b0:b0 + BC], slen[b0:b0 + BC], consts, T))

    res = bass_utils.run_bass_kernel_spmd(nc, in_maps, core_ids=list(range(NCORES)))
    parts = np.concatenate([r["loss"].reshape(-1) for r in res.results])
    return np.float32(parts.mean())


if __name__ == "__main__":
    import jax
    import reference as R
    cpu = jax.devices("cpu")[0]
    with jax.default_device(cpu):
        inputs = {k: np.asarray(jax.device_put(v, cpu)) for k, v in R.setup_inputs().items()}
        expected = float(R.reference(**{k: jax.device_put(v, cpu) for k, v in inputs.items()}))
    got = kernel(**inputs)
    rel = abs(got - expected) / abs(expected)
    print("expected:", expected, "got:", got, "rel:", rel)



# revision 9
# speedup vs baseline: 1.0252x; 1.0252x over previous
"""BiLSTM-CRF loss kernel for 8 Trainium2 NeuronCores (data-parallel over batch).

Self-contained: hardcodes all shapes from the problem spec.
Returns scalar f32 loss (mean over batch of CRF NLL).

Math reformulation (validated vs reference):
 - LSTM gates via one tanh (sigmoid(x) = 0.5 tanh(x/2) + 0.5); i,f,o weight
   rows pre-halved on host. States kept as c' = 2c, h' = 2h (weights absorb).
 - Reverse-direction masking: add -30000 to i,f,o pre-activations at padded
   steps (forces sigmoids to exactly 0 => state resets).
 - Embedding gather: rows padded to 128 cols; indirect-DMA gather to a
   row-per-partition staging tile, then XBAR DMA transpose into time-major
   xT (no PE/vector involvement).
 - LayerNorm folded into the feature matmul; CRF in exp space with kappa
   damping folded into the transition matrix.
 - CRF split at t=m: forward alpha chain covers t<=m (END-readouts batched
   at the end from the w-history), backward beta chain covers t>m with
   per-step masked reset to exp(trans[:,END]) (handles variable sen_len).
   Both chains run interleaved; sequences with len<=m take the alpha path,
   longer ones the alpha_m . beta_m dot at the cut.  Batch is packed two
   32-seq groups per 64 partitions with block-diagonal transition mats.
"""

import numpy as np
import ml_dtypes

VOCAB, EMBD, HID, K = 100000, 50, 200, 32
H = 100
START, END = 30, 31
B, T = 512, 256
NCORES = 8
BC = B // NCORES            # 64 sequences per core
HB = BC // 2                # 32 = CRF batch group size
LN_EPS = 1e-5
KLOG = 4.9                  # -log(kappa)
MCUT = 144                  # CRF forward/backward split point
EP = 128                    # padded embedding row length

bf16 = ml_dtypes.bfloat16

_PROGRAM_CACHE = {}
DEBUG_DUMP = False


def _build_program(Tn):
    import concourse.bass as bass
    import concourse.bacc as bacc
    import concourse.mybir as mybir
    import concourse.tile as tile
    from concourse.alu_op_type import AluOpType as op
    from contextlib import ExitStack

    dt = mybir.dt
    AF = mybir.ActivationFunctionType
    NT = Tn * BC
    CH = NT // 512              # 512-token chunks
    QT = Tn // 4                # timesteps per h-quarter tile
    NB = NT // 128              # 128-token gather blocks
    m = MCUT

    nc = bacc.Bacc()

    d_emb = nc.dram_tensor("emb_tab", [VOCAB, EP], dt.bfloat16, kind="ExternalInput")
    d_gidx = nc.dram_tensor("gidx", [128, NB], dt.int32, kind="ExternalInput")
    d_wx = {dn: nc.dram_tensor(f"wx_{dn}", [66, 4 * H], dt.bfloat16, kind="ExternalInput")
            for dn in "fb"}
    d_wh = {dn: nc.dram_tensor(f"wh_{dn}", [H, 4 * H], dt.bfloat16, kind="ExternalInput")
            for dn in "fb"}
    d_oi = nc.dram_tensor("onesinv", [2, NT], dt.bfloat16, kind="ExternalInput")
    d_wgf = nc.dram_tensor("wgt_f", [H, K], dt.bfloat16, kind="ExternalInput")
    d_wgb = nc.dram_tensor("wgt_b", [H, K], dt.bfloat16, kind="ExternalInput")
    d_nws = nc.dram_tensor("negwsum", [1, K], dt.bfloat16, kind="ExternalInput")
    d_c0 = nc.dram_tensor("c0col", [K, 1], dt.float32, kind="ExternalInput")
    d_mmF = nc.dram_tensor("mmatF2", [BC, BC], dt.bfloat16, kind="ExternalInput")
    d_mmB = nc.dram_tensor("mmatB2", [BC, BC], dt.bfloat16, kind="ExternalInput")
    d_c0r = nc.dram_tensor("c0rep", [BC, HB], dt.bfloat16, kind="ExternalInput")
    d_c0B = nc.dram_tensor("c0B", [BC, 2], dt.bfloat16, kind="ExternalInput")
    d_onB = nc.dram_tensor("onesB", [BC, 2], dt.bfloat16, kind="ExternalInput")
    d_msk = nc.dram_tensor("maskB", [BC, Tn * HB], dt.uint8, kind="ExternalInput")
    d_w0 = nc.dram_tensor("w0p", [BC, HB], dt.bfloat16, kind="ExternalInput")
    d_trT = nc.dram_tensor("transT", [K, K], dt.bfloat16, kind="ExternalInput")
    d_tend = nc.dram_tensor("trans_end", [K, 1], dt.bfloat16, kind="ExternalInput")
    d_c0b = nc.dram_tensor("c0bf", [K, 1], dt.bfloat16, kind="ExternalInput")
    d_ohe = nc.dram_tensor("oh_end", [K, BC], dt.bfloat16, kind="ExternalInput")
    d_ohem = nc.dram_tensor("oh_em", [K, NT], dt.bfloat16, kind="ExternalInput")
    d_ohpr = nc.dram_tensor("oh_prev", [K, NT], dt.bfloat16, kind="ExternalInput")
    d_ui = nc.dram_tensor("u_idx", [BC, 1], dt.int32, kind="ExternalInput")
    d_lenk = nc.dram_tensor("len_klog", [BC, 1], dt.float32, kind="ExternalInput")
    d_loss = nc.dram_tensor("loss", [BC, 1], dt.float32, kind="ExternalOutput")
    if DEBUG_DUMP:
        d_dbg_hf = nc.dram_tensor("dbg_hf", [H, NT], dt.bfloat16, kind="ExternalOutput")
        d_dbg_hb = nc.dram_tensor("dbg_hb", [H, NT], dt.bfloat16, kind="ExternalOutput")
        d_dbg_e = nc.dram_tensor("dbg_e", [BC, Tn * HB], dt.bfloat16, kind="ExternalOutput")
        d_dbg_w = nc.dram_tensor("dbg_w", [BC, (m + 1) * HB], dt.bfloat16, kind="ExternalOutput")
        d_dbg_u = nc.dram_tensor("dbg_u", [(m + 2) * BC, 1], dt.float32, kind="ExternalOutput")

    with tile.TileContext(nc) as tc, ExitStack() as ctx:
        const = ctx.enter_context(tc.tile_pool(name="const", bufs=1))
        big = ctx.enter_context(tc.tile_pool(name="big", bufs=1))
        dramp = ctx.enter_context(tc.tile_pool(name="dramp", bufs=1, space="DRAM"))

        u_d = dramp.tile([(m + 2) * BC, 1], dt.float32, tag="u_d")
        r_d = dramp.tile([BC, 1], dt.float32, tag="r_d")

        wx = {dn: const.tile([66, 4 * H], dt.bfloat16, tag=f"wx{dn}", name=f"wx{dn}") for dn in "fb"}
        wh = {dn: const.tile([H, 4 * H], dt.bfloat16, tag=f"wh{dn}", name=f"wh{dn}") for dn in "fb"}
        for dn in "fb":
            nc.sync.dma_start(wx[dn][:], d_wx[dn][:])
            nc.sync.dma_start(wh[dn][:], d_wh[dn][:])
        wgf = const.tile([H, K], dt.bfloat16)
        wgb = const.tile([H, K], dt.bfloat16)
        nc.sync.dma_start(wgf[:], d_wgf[:])
        nc.sync.dma_start(wgb[:], d_wgb[:])
        nws = const.tile([1, K], dt.bfloat16)
        nc.sync.dma_start(nws[:], d_nws[:])
        c0col = const.tile([K, 1], dt.float32)
        nc.sync.dma_start(c0col[:], d_c0[:])
        mmF = const.tile([BC, BC], dt.bfloat16)
        nc.sync.dma_start(mmF[:], d_mmF[:])
        mmB = const.tile([BC, BC], dt.bfloat16)
        nc.sync.dma_start(mmB[:], d_mmB[:])
        c0rep = const.tile([BC, HB], dt.bfloat16)
        nc.sync.dma_start(c0rep[:], d_c0r[:])
        c0B = const.tile([BC, 2], dt.bfloat16)
        nc.sync.dma_start(c0B[:], d_c0B[:])
        onesB = const.tile([BC, 2], dt.bfloat16)
        nc.sync.dma_start(onesB[:], d_onB[:])
        maskB = const.tile([BC, Tn * HB], dt.uint8)
        nc.sync.dma_start(maskB[:], d_msk[:])
        ones100 = const.tile([H, 1], dt.bfloat16)
        nc.vector.memset(ones100[:], 1.0)
        ones1k = const.tile([1, K], dt.bfloat16)
        nc.vector.memset(ones1k[:], 1.0)
        ones1kf = const.tile([K, 1], dt.bfloat16)
        nc.vector.memset(ones1kf[:], 1.0)

        # persistent h history, [100, NT] per direction as 4 quarter tiles
        hq = {dn: [big.tile([H, QT * BC], dt.bfloat16, tag=f"h{dn}{q}", name=f"h{dn}{q}")
                   for q in range(4)] for dn in "fb"}
        # packed emission exps for the CRF: [64, Tn*32]
        epk = big.tile([BC, Tn * HB], dt.bfloat16, tag="epk", name="epk")
        # forward CRF w history
        whist = big.tile([BC, (m + 1) * HB], dt.bfloat16, tag="whist", name="whist")
        # time-major embeddings (rows 64=invm, 65=ones; 50..63,66..127 zero)
        xT = big.tile([128, NT], dt.bfloat16, tag="xT", name="xT")

        # ================ P0: embedding gather (gpsimd + DMA only) ========
        with tc.tile_pool(name="p0", bufs=4) as p0:
            gidx = const.tile([128, NB], dt.int32)
            nc.sync.dma_start(gidx[:], d_gidx[:])
            # alternate front (fwd) and back (bwd) blocks
            border = []
            for k in range((NB + 1) // 2):
                border.append(k)
                if NB - 1 - k > k:
                    border.append(NB - 1 - k)
            for bi in border:
                xg = p0.tile([128, EP], dt.bfloat16, tag="xg")
                nc.gpsimd.indirect_dma_start(
                    out=xg[:],
                    out_offset=None,
                    in_=d_emb[:],
                    in_offset=bass.IndirectOffsetOnAxis(
                        ap=gidx[:, bi:bi + 1], axis=0),
                )
                dst = bi * 128
                nc.sync.dma_start_transpose(out=xT[:, dst:dst + 128], in_=xg[:])
                nc.sync.dma_start(out=xT[64:66, dst:dst + 128],
                                  in_=d_oi[:, dst:dst + 128])

        # ================ P1: the two LSTMs ================
        with tc.tile_pool(name="p1", bufs=2) as p1, \
             tc.tile_pool(name="p1s", bufs=1) as p1s, \
             tc.tile_pool(name="p1ps", bufs=2, space="PSUM") as p1ps:
            cst = {dn: p1s.tile([H, BC], dt.float32, tag=f"c{dn}", name=f"c{dn}") for dn in "fb"}
            psb = {dn: None for dn in "fb"}   # current 4-step psum block

            def lstm_step(dn, t, prev_t, first):
                blk, sl = t // 4, t % 4
                lead_sl = 3 if dn == "b" else 0   # first-consumed slot of a block
                last_sl = 0 if dn == "b" else 3
                if sl == lead_sl:
                    ps = p1ps.tile([H, 1024], dt.float32, tag=f"ps{dn}", name=f"ps{dn}")
                    psb[dn] = ps
                    rx = xT[0:66, blk * 256:(blk + 1) * 256]
                    for g in range(4):
                        nc.tensor.matmul(out=ps[:, g * 256:(g + 1) * 256],
                                         lhsT=wx[dn][:, g * H:(g + 1) * H], rhs=rx,
                                         start=True, stop=False, skip_group_check=True)
                ps = psb[dn]
                if not first:
                    pq, pc = prev_t // QT, (prev_t % QT) * BC
                    rh = hq[dn][pq][:, pc:pc + BC]
                    for g in range(4):
                        nc.tensor.matmul(
                            out=ps[:, g * 256 + sl * 64:g * 256 + sl * 64 + 64],
                            lhsT=wh[dn][:, g * H:(g + 1) * H], rhs=rh,
                            start=False, stop=(sl == last_sl and g == 3),
                            skip_group_check=True)
                # gates for this step: [100, 4, 64] strided view of the block
                gv = ps[:].rearrange("p (g s b) -> p g (s b)", g=4, s=4)[
                    :, :, sl * 64:(sl + 1) * 64]
                G = p1.tile([H, 256], dt.bfloat16, tag=f"G{dn}")
                nc.scalar.activation(out=G[:].rearrange("p (g b) -> p g b", g=4),
                                     in_=gv, func=AF.Tanh)
                th_i, th_f = G[:, 0:64], G[:, 64:128]
                th_o, th_g = G[:, 128:192], G[:, 192:256]
                c = cst[dn]
                u = p1.tile([H, BC], dt.bfloat16, tag=f"u{dn}")
                nc.vector.scalar_tensor_tensor(out=u[:], in0=th_i, scalar=1.0,
                                               in1=th_g, op0=op.add, op1=op.mult)
                if first:
                    nc.vector.tensor_copy(out=c[:], in_=u[:])
                else:
                    v = p1.tile([H, BC], dt.float32, tag=f"v{dn}")
                    nc.vector.scalar_tensor_tensor(out=v[:], in0=th_f, scalar=1.0,
                                                   in1=c[:], op0=op.add, op1=op.mult)
                    nc.vector.scalar_tensor_tensor(out=c[:], in0=v[:], scalar=0.5,
                                                   in1=u[:], op0=op.mult, op1=op.add)
                thc = p1.tile([H, BC], dt.bfloat16, tag=f"thc{dn}")
                nc.scalar.activation(out=thc[:], in_=c[:], func=AF.Tanh, scale=0.5)
                qh, ch_ = t // QT, (t % QT) * BC
                nc.vector.scalar_tensor_tensor(
                    out=hq[dn][qh][:, ch_:ch_ + BC], in0=th_o, scalar=1.0,
                    in1=thc[:], op0=op.add, op1=op.mult)

            for s in range(Tn):
                lstm_step("f", s, s - 1, s == 0)
                lstm_step("b", Tn - 1 - s, Tn - s, s == 0)

        # ================ P2: LN stats + feats + gold score ================
        PKC = NT // 128
        goldctx = ExitStack()
        goldps = goldctx.enter_context(tc.tile_pool(name="goldps", bufs=1, space="PSUM"))
        with tc.tile_pool(name="p2", bufs=3) as p2, \
             tc.tile_pool(name="p2s", bufs=1) as p2s, \
             tc.tile_pool(name="p2ps", bufs=1, space="PSUM") as p2ps, \
             tc.tile_pool(name="p2pg", bufs=2, space="PSUM") as p2pg:

            mupk = p2s.tile([128, PKC], dt.float32, tag="mupk")
            msqpk = p2s.tile([128, PKC], dt.float32, tag="msqpk")
            for c in range(CH):
                psmu = p2ps.tile([1, 512], dt.float32, tag="psmu")
                psmsq = p2ps.tile([1, 512], dt.float32, tag="psmsq")
                q, off = (c * 512) // (QT * BC), (c * 512) % (QT * BC)
                hfc = hq["f"][q][:, off:off + 512]
                hbc = hq["b"][q][:, off:off + 512]
                hsqf = p2.tile([H, 512], dt.bfloat16, tag="hsqf")
                nc.vector.tensor_tensor(out=hsqf[:], in0=hfc, in1=hfc, op=op.mult)
                hsqb = p2.tile([H, 512], dt.bfloat16, tag="hsqb")
                nc.vector.tensor_tensor(out=hsqb[:], in0=hbc, in1=hbc, op=op.mult)
                nc.tensor.matmul(out=psmu[:], lhsT=ones100[:], rhs=hfc,
                                 start=True, stop=False)
                nc.tensor.matmul(out=psmu[:], lhsT=ones100[:], rhs=hbc,
                                 start=False, stop=True)
                nc.tensor.matmul(out=psmsq[:], lhsT=ones100[:],
                                 rhs=hsqf[:], start=True, stop=False)
                nc.tensor.matmul(out=psmsq[:], lhsT=ones100[:],
                                 rhs=hsqb[:], start=False, stop=True)
                stgmu = p2.tile([1, 512], dt.float32, tag="stgmu")
                nc.scalar.copy(out=stgmu[:], in_=psmu[:])
                stgmsq = p2.tile([1, 512], dt.float32, tag="stgmsq")
                nc.vector.tensor_copy(out=stgmsq[:], in_=psmsq[:])
                nc.sync.dma_start(out=mupk[4 * c:4 * c + 4, :], in_=stgmu[:])
                nc.sync.dma_start(out=msqpk[4 * c:4 * c + 4, :], in_=stgmsq[:])
            sq = p2s.tile([128, PKC], dt.float32, tag="sqpk")
            nc.vector.scalar_tensor_tensor(out=sq[:], in0=mupk[:], scalar=1.0 / 160000.0,
                                           in1=mupk[:], op0=op.mult, op1=op.mult)
            var = p2s.tile([128, PKC], dt.float32, tag="varpk")
            nc.vector.scalar_tensor_tensor(out=var[:], in0=msqpk[:], scalar=1.0 / 800.0,
                                           in1=sq[:], op0=op.mult, op1=op.subtract)
            epsc = p2s.tile([128, 1], dt.float32, tag="epsc")
            nc.vector.memset(epsc[:], LN_EPS)
            lnv = p2s.tile([128, PKC], dt.float32, tag="lnvpk")
            nc.scalar.activation(out=lnv[:], in_=var[:], func=AF.Ln, bias=epsc[:])
            rstdpk = p2s.tile([128, PKC], dt.bfloat16, tag="rstdpk")
            nc.scalar.activation(out=rstdpk[:], in_=lnv[:], func=AF.Exp, scale=-0.5)
            spk = p2s.tile([128, PKC], dt.bfloat16, tag="spk")
            nc.vector.tensor_copy(out=spk[:], in_=mupk[:])

            trT = p2s.tile([K, K], dt.bfloat16, tag="trT")
            nc.sync.dma_start(trT[:], d_trT[:])
            c0b = p2s.tile([K, 1], dt.bfloat16, tag="c0b")
            nc.sync.dma_start(c0b[:], d_c0b[:])
            realp = goldps.tile([1, 512], dt.float32, tag="realp")
            n_acc = 3 * CH
            k_acc = 0
            for c in range(CH):
                pg = p2pg.tile([K, 512], dt.float32, tag="pg")
                q, off = (c * 512) // (QT * BC), (c * 512) % (QT * BC)
                nc.tensor.matmul(out=pg[:], lhsT=wgf[:], rhs=hq["f"][q][:, off:off + 512],
                                 start=True, stop=False)
                nc.tensor.matmul(out=pg[:], lhsT=wgb[:], rhs=hq["b"][q][:, off:off + 512],
                                 start=False, stop=False)
                rstg = p2.tile([1, 512], dt.bfloat16, tag="rstg")
                nc.sync.dma_start(out=rstg[:], in_=rstdpk[4 * c:4 * c + 4, :])
                sstg = p2.tile([1, 512], dt.bfloat16, tag="sstg")
                nc.sync.dma_start(out=sstg[:], in_=spk[4 * c:4 * c + 4, :])
                nc.tensor.matmul(out=pg[:], lhsT=nws[:], rhs=sstg[:],
                                 start=False, stop=True)
                rb = p2pg.tile([K, 512], dt.float32, tag="rb", bufs=1)
                nc.tensor.matmul(out=rb[:], lhsT=ones1k[:], rhs=rstg[:],
                                 start=True, stop=True)
                rbs = p2.tile([K, 512], dt.bfloat16, tag="rbs")
                nc.scalar.copy(out=rbs[:], in_=rb[:])
                fsl = p2.tile([K, 512], dt.bfloat16, tag="fsl")
                nc.vector.tensor_tensor(out=fsl[:], in0=pg[:], in1=rbs[:], op=op.mult)
                # emission exps into the packed CRF layout (two 32-batch groups)
                fv = fsl[:].rearrange("p (t b) -> p t b", t=8)
                evA = epk[0:K, c * 256:(c + 1) * 256]
                evB = epk[K:BC, c * 256:(c + 1) * 256]
                nc.scalar.activation(
                    out=evA.rearrange("p (t b) -> p t b", t=8),
                    in_=fv[:, :, 0:HB], func=AF.Exp, bias=c0col[:])
                nc.scalar.activation(
                    out=evB.rearrange("p (t b) -> p t b", t=8),
                    in_=fv[:, :, HB:BC], func=AF.Exp, bias=c0col[:])
                # ---- gold-score accumulation for this chunk ----
                ohem = p2.tile([K, 512], dt.bfloat16, tag="ohem")
                nc.sync.dma_start(ohem[:], d_ohem[:, c * 512:(c + 1) * 512])
                ohpr = p2.tile([K, 512], dt.bfloat16, tag="ohpr")
                nc.sync.dma_start(ohpr[:], d_ohpr[:, c * 512:(c + 1) * 512])
                Rp = p2pg.tile([K, 512], dt.float32, tag="Rp", bufs=1)
                nc.tensor.matmul(out=Rp[:], lhsT=trT[:], rhs=ohem[:],
                                 start=True, stop=True)
                tsel = p2.tile([K, 512], dt.bfloat16, tag="tsel")
                nc.vector.tensor_tensor(out=tsel[:], in0=Rp[:], in1=ohpr[:], op=op.mult)
                esel = p2.tile([K, 512], dt.bfloat16, tag="esel")
                nc.vector.tensor_tensor(out=esel[:], in0=fsl[:], in1=ohem[:], op=op.mult)
                for rhs_ in (tsel[:], esel[:]):
                    nc.tensor.matmul(out=realp[:], lhsT=ones1kf[:], rhs=rhs_,
                                     start=(k_acc == 0), stop=(k_acc == n_acc - 1))
                    k_acc += 1
                nc.tensor.matmul(out=realp[:], lhsT=c0b[:], rhs=ohem[:],
                                 start=(k_acc == 0), stop=(k_acc == n_acc - 1))
                k_acc += 1

        # ================ P3: CRF fwd/bwd recursions ================
        with tc.tile_pool(name="p3", bufs=1) as p3, \
             tc.tile_pool(name="p3ps", bufs=2, space="PSUM") as p3ps:
            nc.sync.dma_start(whist[:, 0:HB], d_w0[:])
            ub = [p3.tile([BC, HB], dt.bfloat16, tag=f"ub{i}", name=f"ub{i}")
                  for i in range(2)]
            # bwd init: u'_Tn = e_Tn * c0
            nc.vector.tensor_tensor(out=ub[Tn % 2][:],
                                    in0=epk[:, (Tn - 1) * HB:Tn * HB],
                                    in1=c0rep[:], op=op.mult)

            def fwd_step(t):
                pv = p3ps.tile([BC, HB], dt.float32, tag="pvF", bufs=2)
                nc.tensor.matmul(out=pv[:], lhsT=mmF[:],
                                 rhs=whist[:, (t - 1) * HB:t * HB],
                                 start=True, stop=True)
                nc.vector.tensor_tensor(out=whist[:, t * HB:(t + 1) * HB],
                                        in0=pv[:],
                                        in1=epk[:, (t - 1) * HB:t * HB],
                                        op=op.mult)

            def bwd_step(t):
                # v_t = mask? c0 : MB @ u'_{t+1};  u'_t = e_t * v_t
                pv = p3ps.tile([BC, HB], dt.float32, tag="pvB", bufs=2)
                nc.tensor.matmul(out=pv[:], lhsT=mmB[:], rhs=ub[(t + 1) % 2][:],
                                 start=True, stop=True)
                nc.vector.copy_predicated(out=pv[:],
                                          mask=maskB[:, (t - 1) * HB:t * HB],
                                          data=c0rep[:])
                return pv

            for i in range(Tn - m):          # bwd t = Tn-1 .. m+1
                t_b = Tn - 1 - i
                if i < m:                    # fwd t = 1 .. Tn-m
                    fwd_step(i + 1)
                if t_b >= m + 1:
                    pv = bwd_step(t_b)
                    nc.vector.tensor_tensor(out=ub[t_b % 2][:], in0=pv[:],
                                            in1=epk[:, (t_b - 1) * HB:t_b * HB],
                                            op=op.mult)
            for t in range(Tn - m + 1, m + 1):   # remaining fwd steps
                fwd_step(t)
            # cut-point combine for len > m: dot(alpha_m, beta_m)
            pvm = bwd_step(m)
            dtm = p3.tile([BC, HB], dt.bfloat16, tag="dtm")
            nc.vector.tensor_tensor(out=dtm[:], in0=pvm[:],
                                    in1=whist[:, m * HB:(m + 1) * HB], op=op.mult)
            dps = p3ps.tile([2, HB], dt.float32, tag="dps", bufs=1)
            nc.tensor.matmul(out=dps[:], lhsT=onesB[:], rhs=dtm[:],
                             start=True, stop=True)
            dsb = p3.tile([2, HB], dt.float32, tag="dsb")
            nc.scalar.copy(out=dsb[:], in_=dps[:])
            nc.sync.dma_start(out=u_d[(m + 1) * BC:(m + 2) * BC, :], in_=dsb[:])
            # batched END-readouts for the fwd chain: u_t = c0 . w_t
            NW = (m + 1) * HB
            usb = p3.tile([2, NW], dt.float32, tag="usb")
            for j0 in range(0, NW, 512):
                j1 = min(j0 + 512, NW)
                ups = p3ps.tile([2, j1 - j0], dt.float32, tag="ups", bufs=2)
                nc.tensor.matmul(out=ups[:], lhsT=c0B[:],
                                 rhs=whist[:, j0:j1], start=True, stop=True)
                nc.scalar.copy(out=usb[:, j0:j1], in_=ups[:])
            nc.sync.dma_start(
                out=u_d[0:(m + 1) * BC, :].rearrange(
                    "(t g b) one -> g t (b one)", g=2, b=HB),
                in_=usb[:])

        if DEBUG_DUMP:
            for q in range(4):
                nc.sync.dma_start(d_dbg_hf[:, q * QT * BC:(q + 1) * QT * BC], hq["f"][q][:])
                nc.sync.dma_start(d_dbg_hb[:, q * QT * BC:(q + 1) * QT * BC], hq["b"][q][:])
            nc.sync.dma_start(d_dbg_e[:], epk[:])
            nc.sync.dma_start(d_dbg_w[:], whist[:])

        # ================ P4: final loss ================
        with tc.tile_pool(name="p4", bufs=1) as p4, \
             tc.tile_pool(name="p4ps", bufs=1, space="PSUM") as p4ps:
            tend = p4.tile([K, 1], dt.bfloat16, tag="tend")
            nc.sync.dma_start(tend[:], d_tend[:])
            ohe = p4.tile([K, BC], dt.bfloat16, tag="ohe")
            nc.sync.dma_start(ohe[:], d_ohe[:])
            endp = p4ps.tile([1, BC], dt.float32, tag="endp")
            nc.tensor.matmul(out=endp[:], lhsT=tend[:], rhs=ohe[:],
                             start=True, stop=True)
            rsub = p4.tile([1, BC], dt.float32, tag="rsub")
            nc.vector.tensor_reduce(
                out=rsub[:], in_=realp[:].rearrange("one (t b) -> one b t", b=BC),
                axis=mybir.AxisListType.X, op=op.add)
            rrow2 = p4.tile([1, BC], dt.float32, tag="rrow2")
            nc.vector.tensor_tensor(out=rrow2[:], in0=rsub[:], in1=endp[:], op=op.add)
            nc.sync.dma_start(out=r_d[:], in_=rrow2[:])
            rcol = p4.tile([BC, 1], dt.float32, tag="rcol")
            nc.sync.dma_start(out=rcol[:], in_=r_d[:])

            ui = p4.tile([BC, 1], dt.int32, tag="ui")
            nc.sync.dma_start(ui[:], d_ui[:])
            lenk = p4.tile([BC, 1], dt.float32, tag="lenk")
            nc.sync.dma_start(lenk[:], d_lenk[:])
            ug = p4.tile([BC, 1], dt.float32, tag="ug")
            nc.gpsimd.indirect_dma_start(out=ug[:], out_offset=None, in_=u_d[:],
                                         in_offset=bass.IndirectOffsetOnAxis(ap=ui[:], axis=0))
            if DEBUG_DUMP:
                nc.sync.dma_start(d_dbg_u[:], u_d[:])
            tot = p4.tile([BC, 1], dt.float32, tag="tot")
            nc.scalar.activation(out=tot[:], in_=ug[:], func=AF.Ln)
            nc.vector.tensor_tensor(out=tot[:], in0=tot[:], in1=lenk[:], op=op.add)
            lout = p4.tile([BC, 1], dt.float32, tag="lout")
            nc.vector.tensor_tensor(out=lout[:], in0=tot[:], in1=rcol[:], op=op.subtract)
            nc.sync.dma_start(out=d_loss[:], in_=lout[:])
        goldctx.close()

    nc.compile()
    return nc


def _prep_core_inputs(sent, tags, slen, consts, Tn):
    """Host-side index prep for one core. sent/tags [BC,Tn] slen [BC]."""
    NT = Tn * BC
    m = MCUT

    sent_tm = np.ascontiguousarray(sent.T).reshape(-1)      # t-major tokens
    gidx = np.ascontiguousarray(sent_tm.reshape(NT // 128, 128).T).astype(np.int32)

    tgrid = np.repeat(np.arange(Tn), BC)
    bgrid = np.tile(np.arange(BC), Tn)
    invm = (tgrid >= slen[bgrid]).astype(np.float32)
    onesinv = np.stack([invm, np.ones(NT, np.float32)]).astype(bf16)  # rows 64,65

    tags_ext = np.concatenate([np.full((BC, 1), START, np.int64), tags], axis=1)
    mrow = (tgrid < slen[bgrid]).astype(np.float32)          # [NT] mask, t-major
    tag_tm = tags.T.reshape(-1)                              # tag at token (t,b)
    prev_tm = tags_ext[:, :Tn].T.reshape(-1)                 # prev tag at (t,b)
    kk = np.arange(K)[:, None]
    oh_em = ((tag_tm[None, :] == kk) * mrow[None, :]).astype(bf16)
    oh_prev = ((prev_tm[None, :] == kk) * mrow[None, :]).astype(bf16)
    oh_end = (tags_ext[np.arange(BC), slen][None, :] == kk).astype(bf16)

    # CRF packed-group tables
    # maskB[k(2 groups), (t-1)*32+j] = 1.0 where t >= len of batch (g*32+j)
    tt = np.arange(1, Tn + 1)[:, None]                       # step index t
    mA = (tt >= slen[None, 0:HB]).astype(np.float32)         # [Tn, 32]
    mB_ = (tt >= slen[None, HB:BC]).astype(np.float32)
    maskB = np.zeros((BC, Tn * HB), np.float32)
    maskB[0:32, :] = np.broadcast_to(mA.reshape(1, Tn * HB), (32, Tn * HB))
    maskB[32:64, :] = np.broadcast_to(mB_.reshape(1, Tn * HB), (32, Tn * HB))

    w0p = np.zeros((BC, HB), np.float32)
    w0p[START, :] = 1.0
    w0p[32 + START, :] = 1.0

    ui = np.where(slen <= m, slen * BC + np.arange(BC),
                  (m + 1) * BC + np.arange(BC)).astype(np.int32).reshape(BC, 1)

    d = dict(consts)
    d.update(dict(
        gidx=gidx,
        onesinv=np.ascontiguousarray(onesinv),
        oh_em=np.ascontiguousarray(oh_em),
        oh_prev=np.ascontiguousarray(oh_prev),
        oh_end=np.ascontiguousarray(oh_end),
        maskB=np.ascontiguousarray(maskB).astype(np.uint8),
        w0p=w0p.astype(bf16),
        u_idx=ui,
        len_klog=(slen * KLOG).astype(np.float32).reshape(BC, 1),
    ))
    return d


def _prep_consts(emb, Wf_ih, Wf_hh, bfv, Wb_ih, Wb_hh, bbv, gamma, beta, W_lin, trans, Tn):
    sc = np.ones((4 * H, 1), np.float32)
    sc[0:H] = 0.5
    sc[H:2 * H] = 0.5
    sc[3 * H:4 * H] = 0.5
    # reference gate order [i,f,g,o] -> device order [i,f,o,g]
    perm = np.concatenate([np.arange(0, H), np.arange(H, 2 * H),
                           np.arange(3 * H, 4 * H), np.arange(2 * H, 3 * H)])

    def mk(Wi, Wh, b, bwd):
        Wi_s, Wh_s, b_s = Wi * sc, Wh * sc * 0.5, b * sc[:, 0]
        Wi_p, Wh_p, b_p = Wi_s[perm], Wh_s[perm], b_s[perm]
        wxa = np.zeros((66, 4 * H), np.float32)
        wxa[:EMBD] = Wi_p.T
        wxa[65] = b_p
        if bwd:
            wxa[64, 0:3 * H] = -30000.0   # i, f, o gate masking via invm row
        return np.ascontiguousarray(wxa).astype(bf16), \
            np.ascontiguousarray(Wh_p.T).astype(bf16)

    wx_f, wh_f = mk(Wf_ih, Wf_hh, bfv, False)
    wx_b, wh_b = mk(Wb_ih, Wb_hh, bbv, True)

    Wg = (W_lin * gamma[None, :]) * 0.5
    wsum = (W_lin * gamma[None, :]).sum(1)
    c0 = (W_lin @ beta).astype(np.float32)
    kap = np.exp(-KLOG)
    mm1 = kap * np.exp(trans)                    # [K,K], [k, m] for fwd
    mmatF2 = np.zeros((BC, BC), np.float32)
    mmatF2[0:K, 0:K] = mm1
    mmatF2[K:2 * K, K:2 * K] = mm1
    mmatB2 = np.zeros((BC, BC), np.float32)
    mmatB2[0:K, 0:K] = mm1.T                     # [j, k] = kap*exp(trans[k,j])
    mmatB2[K:2 * K, K:2 * K] = mm1.T
    c0v = np.exp(trans[:, END])
    c0rep = np.tile(c0v.reshape(K, 1), (2, HB))
    c0B = np.zeros((BC, 2), np.float32)
    c0B[0:K, 0] = c0v
    c0B[K:2 * K, 1] = c0v
    onesB = np.zeros((BC, 2), np.float32)
    onesB[0:K, 0] = 1.0
    onesB[K:2 * K, 1] = 1.0

    emb2 = np.zeros((VOCAB, EP), np.float32)
    emb2[:, :EMBD] = emb

    return dict(
        emb_tab=np.ascontiguousarray(emb2).astype(bf16),
        wx_f=wx_f, wh_f=wh_f, wx_b=wx_b, wh_b=wh_b,
        wgt_f=np.ascontiguousarray(Wg[:, :H].T).astype(bf16),
        wgt_b=np.ascontiguousarray(Wg[:, H:].T).astype(bf16),
        negwsum=np.ascontiguousarray((-(wsum / 400.0)).reshape(1, K)).astype(bf16),
        c0col=np.ascontiguousarray(c0.reshape(K, 1)),
        mmatF2=mmatF2.astype(bf16), mmatB2=mmatB2.astype(bf16),
        c0rep=c0rep.astype(bf16), c0B=c0B.astype(bf16), onesB=onesB.astype(bf16),
        transT=np.ascontiguousarray(trans.T).astype(bf16),
        trans_end=np.ascontiguousarray(trans[:, END].reshape(K, 1)).astype(bf16),
        c0bf=np.ascontiguousarray(c0.reshape(K, 1)).astype(bf16),
    )


def kernel(sentence, tags, sen_len, emb, Wf_ih, Wf_hh, bf, Wb_ih, Wb_hh, bb,
           gamma, beta, W_lin, trans):
    from concourse import bass_utils

    sentence = np.asarray(sentence).astype(np.int64)
    tags_a = np.asarray(tags).astype(np.int64)
    slen = np.asarray(sen_len).astype(np.int64)
    fp = lambda a: np.ascontiguousarray(np.asarray(a), dtype=np.float32)

    consts = _prep_consts(fp(emb), fp(Wf_ih), fp(Wf_hh), fp(bf), fp(Wb_ih), fp(Wb_hh),
                          fp(bb), fp(gamma), fp(beta), fp(W_lin), fp(trans), T)

    if T not in _PROGRAM_CACHE:
        _PROGRAM_CACHE[T] = _build_program(T)
    nc = _PROGRAM_CACHE[T]

    in_maps = []
    for core in range(NCORES):
        b0 = core * BC
        in_maps.append(_prep_core_inputs(
            sentence[b0:b0 + BC], tags_a[b0:b0 + BC], slen[b0:b0 + BC], consts, T))

    res = bass_utils.run_bass_kernel_spmd(nc, in_maps, core_ids=list(range(NCORES)))
    parts = np.concatenate([r["loss"].reshape(-1) for r in res.results])
    return np.float32(parts.mean())


if __name__ == "__main__":
    import jax
    import reference as R
    cpu = jax.devices("cpu")[0]
    with jax.default_device(cpu):
        inputs = {k: np.asarray(jax.device_put(v, cpu)) for k, v in R.setup_inputs().items()}
        expected = float(R.reference(**{k: jax.device_put(v, cpu) for k, v in inputs.items()}))
    got = kernel(**inputs)
    rel = abs(got - expected) / abs(expected)
    print("expected:", expected, "got:", got, "rel:", rel)
